# revision 28
# baseline (speedup 1.0000x reference)
"""Trainium2 Bass kernel for nn_MaxExtractor (masked pairwise-IoU max + union max).

Contract: kernel(**inputs) takes FULL unsharded inputs, returns the FULL [2]
output. Internally shards the batch dim (8 images) across 8 NeuronCores, one
image per core; each core computes [max_prob_t, iou_max_of_its_image]; the
host gathers and averages the per-image iou scalars.

v2 design (per core, N=4096 preds, M=2048 gts, K=64 person slots):
  - All coordinates are shipped as fp16, centered by -IMG/2 so the fp16
    quantization step is <=0.25 over the whole canvas.
  - gt coord column-planes live in DRAM as [nhalf, 4*GTW]; ONE broadcast DMA
    (stride-0 partition dim) replicates each half across 64 partitions
    directly into SBUF [128, 4*GTW] fp16 - no PE broadcast, no PSUM.
  - Person compaction: class mask -> free-dim cumsum -> cross-partition
    prefix (triangular fp16 matmul) -> ranks -> 32x one-hot (fp16, 4x DVE
    mode) -> accumulating PE matmuls gather person boxes into [128, 4].
  - Pairwise runs all-SBUF fp16 (tensor_scalar at 4x, tensor_tensor at 2x):
    t1x = min(gx2,px2), m2x = max(gx1,px1), zx = t1x-m2x, same for y,
    inter = relu(zx)*relu(zy).
  - Ranking in log domain on the otherwise-idle Act engine:
    lnd = Ln(inter) - Ln(area_p + area_g), max-reduced; the single winner is
    mapped back via r = exp(lnd), iou = r/(1-r).  Areas for gt come from the
    broadcast planes (3 fp16 tensor_tensor ops).
  - Union max: fp16 mask-multiply + reduce.
  - Final: gpsimd partition all-reduce(max) over [umax | iou], 1 output DMA.
"""

import sys

sys.path.insert(0, "/opt/trn_rl_repo")

import contextlib

import numpy as np

import concourse.bacc as bacc
import concourse.mybir as mybir
from concourse import bass_isa
from concourse.tile import TileContext

F32 = mybir.dt.float32
F16 = mybir.dt.float16
U32 = mybir.dt.uint32
I32 = mybir.dt.int32
Alu = mybir.AluOpType
Act = mybir.ActivationFunctionType

N = 4096  # preds per image
M = 2048  # gts per image
B = 8  # images == cores
U = 4096  # union entries
CEN = 320.0  # coordinate centering offset (IMG/2)
EPS = 1.0e-9
NCH = 32  # pred chunks of 128 (compaction contract dim)
NPOOL_OH = 8  # one-hot chunks built on Pool (rest on DVE)


def build_kernel(K: int):
    """Build the per-core Bass module. K = person-slot count (64 or 128)."""
    assert K in (64, 128)
    nhalf = 128 // K  # gt halves packed along partitions
    GTW = M // nhalf  # gt columns per partition (1024 for K=64)
    HW = GTW // 2  # half-width for the pipelined ln/reduce stage

    nc = bacc.Bacc("TRN2", target_bir_lowering=False, debug=False)

    # host-packed inputs
    #   big u32 [128, 256]: cls(32) | pb_f16(64) | uscore_f16(16) | ucls_f16(16)
    #                      | tri_f16(64) | iota_f16(64)
    #   gt_cols f16 [nhalf, 5*GTW]: x1 | x2 | y1 | y2 | area planes
    big = nc.dram_tensor("big", [128, 256], U32, kind="ExternalInput")
    gt_cols = nc.dram_tensor("gt_cols", [nhalf, 5 * GTW], F16, kind="ExternalInput")
    out = nc.dram_tensor("out", [128, 5], F32, kind="ExternalOutput")

    with TileContext(nc) as tc:
        ctx = contextlib.ExitStack()
        with ctx:
            sb = ctx.enter_context(tc.tile_pool(name="sbuf", bufs=1))
            wrk = ctx.enter_context(tc.tile_pool(name="wrk", bufs=2))
            ohp = ctx.enter_context(tc.tile_pool(name="ohp", bufs=32))
            small = ctx.enter_context(tc.tile_pool(name="small", bufs=1))
            ps_s = ctx.enter_context(tc.tile_pool(name="ps_s", bufs=2, space="PSUM"))

            # ---------------- loads ----------------
            bigt = sb.tile([128, 256], U32, tag="bigt")
            nc.sync.dma_start(out=bigt[:], in_=big.ap())
            cls_sb = bigt[:, 0:32].bitcast(I32)
            pb = bigt[:, 32:96].bitcast(F16)  # [128, 128] = 32 chunks x 4
            uscore = bigt[:, 96:112].bitcast(F16)  # [128, 32]
            ucls = bigt[:, 112:128].bitcast(F16)  # [128, 32]
            tri_sb = bigt[:, 128:192].bitcast(F16)  # [128, 128]
            iota_sb = bigt[:, 192:256].bitcast(F16)  # [128, 128], (j % K) + 1

            # gt broadcast: partition p reads half p // K of gt_cols.
            # Coord planes and the area plane ship as separate DMAs so the
            # pairwise chain can start before the area transfer completes.
            gtb = sb.tile([128, 5 * GTW], F16, tag="gtb")
            src_xy = gt_cols.ap()[:, 0 : 4 * GTW].unsqueeze(1)
            nc.sync.dma_start(
                out=gtb[:, 0 : 4 * GTW],
                in_=src_xy.broadcast_to([nhalf, K, 4 * GTW]),
            )
            src_ag = gt_cols.ap()[:, 4 * GTW : 5 * GTW].unsqueeze(1)
            nc.sync.dma_start(
                out=gtb[:, 4 * GTW : 5 * GTW],
                in_=src_ag.broadcast_to([nhalf, K, GTW]),
            )

            # preload the Ln activation table while DMAs are in flight: a
            # dummy Ln on a memset tile adopts the table-load so the real
            # lnS/lnI calls don't pay the 1283ns load on the critical path
            dmy = small.tile([128, 1], F32, tag="dmy")
            nc.vector.memset(dmy[:], 1.0)
            dmy2 = small.tile([128, 1], F32, tag="dmy2")
            nc.scalar.activation(dmy2[:], dmy[:], Act.Ln)
            gx1 = gtb[:, 0 * GTW : 1 * GTW]
            gx2 = gtb[:, 1 * GTW : 2 * GTW]
            gy1 = gtb[:, 2 * GTW : 3 * GTW]
            gy2 = gtb[:, 3 * GTW : 4 * GTW]
            agb = gtb[:, 4 * GTW : 5 * GTW]

            # ---------------- person mask + ranks (DVE) ----------------
            m = small.tile([128, 32], F16, tag="m")
            nc.vector.tensor_scalar(m[:], cls_sb[:], 0, None, Alu.is_equal)
            s = small.tile([128, 32], F16, tag="s")
            nc.vector.tensor_tensor_scan(s[:], m[:], m[:], 0.0, Alu.add, Alu.max)
            pref_ps = ps_s.tile([128, 4], F32, tag="pss")
            nc.tensor.matmul(
                pref_ps[:, 0:1], tri_sb, s[:, 31:32], start=True, stop=True
            )
            q = small.tile([128, 32], F32, tag="q")
            nc.vector.scalar_tensor_tensor(
                q[:], s[:], pref_ps[:, 0:1], m[:], Alu.add, Alu.mult
            )

            # ---------------- compaction: one-hot + matmul gather -----------
            # oh[p, j] = (q[p, f] == iota[j]), iota[j] = (j % K) + 1
            # Last NPOOL_OH chunks build on Pool so PE's in-order accumulation
            # is never stalled by the slower Pool ops.
            pc_ps = ps_s.tile([128, 4], F32, tag="pss")
            for f in range(NCH):
                oh = ohp.tile([128, 128], F16, tag="oh")
                eng = nc.gpsimd if f < NPOOL_OH else nc.vector
                eng.tensor_scalar(
                    oh[:], iota_sb, q[:, f : f + 1], None, Alu.is_equal
                )
                nc.tensor.matmul(
                    pc_ps[:], oh[:], pb[:, 4 * f : 4 * f + 4],
                    start=(f == 0), stop=(f == NCH - 1),
                )
            # pairwise scalars read straight from PSUM (one PSUM operand per
            # op is legal), so t1x/m2x don't wait on an SBUF copy
            px1, py1, px2, py2 = (pc_ps[:, i : i + 1] for i in range(4))

            # ---------------- pairwise intersection (fp16, all SBUF) --------
            t1x = wrk.tile([128, GTW], F16, tag="t1x")
            nc.vector.tensor_scalar(t1x[:], gx2, px2, None, Alu.min)
            m2x = wrk.tile([128, GTW], F16, tag="m2x")
            nc.vector.tensor_scalar(m2x[:], gx1, px1, None, Alu.max)
            zx = wrk.tile([128, GTW], F16, tag="zx")
            nc.vector.tensor_sub(zx[:], t1x[:], m2x[:])
            t1y = wrk.tile([128, GTW], F16, tag="t1y")
            nc.vector.tensor_scalar(t1y[:], gy2, py2, None, Alu.min)
            m2y = wrk.tile([128, GTW], F16, tag="m2y")
            nc.vector.tensor_scalar(m2y[:], gy1, py1, None, Alu.max)
            zy = wrk.tile([128, GTW], F16, tag="zy")
            nc.vector.tensor_sub(zy[:], t1y[:], m2y[:])

            # pred areas (SBUF copy needed: two-operand ops can't both read
            # PSUM); feeds lnS which is only needed by the first lnd
            pc = small.tile([128, 4], F32, tag="pcs")
            nc.vector.tensor_copy(pc[:], pc_ps[:])
            wp = small.tile([128, 1], F32, tag="wp")
            nc.vector.tensor_sub(wp[:], pc[:, 2:3], pc[:, 0:1])
            hp = small.tile([128, 1], F32, tag="hp")
            nc.vector.tensor_sub(hp[:], pc[:, 3:4], pc[:, 1:2])
            ap_eps = small.tile([128, 1], F32, tag="ap_eps")
            nc.vector.scalar_tensor_tensor(
                ap_eps[:], wp[:], EPS, hp[:], Alu.bypass, Alu.mult
            )
            nc.vector.tensor_scalar(ap_eps[:], ap_eps[:], EPS, None, Alu.add)

            # lnS = Ln(agb + area_p) on Act, in quarters for pipelining
            QW = GTW // 4
            lnS = wrk.tile([128, GTW], F16, tag="lnS")
            for hq in range(4):
                sl = slice(hq * QW, (hq + 1) * QW)
                nc.scalar.activation(
                    lnS[:, sl], agb[:, sl], Act.Ln, bias=ap_eps[:], scale=1.0
                )
            # rx/ry/inter per half so the Act ln chain starts earlier;
            # lnI/lnd/reduce per quarter to pipeline Act against DVE
            rx = wrk.tile([128, GTW], F16, tag="rx")
            ry = wrk.tile([128, GTW], F16, tag="ry")
            inter = wrk.tile([128, GTW], F16, tag="inter")
            lnI = wrk.tile([128, GTW], F16, tag="lnI")
            lnd = wrk.tile([128, GTW], F16, tag="lnd")
            # red cols 0-3: lnd quarter maxima; col 4: union max
            red = small.tile([128, 5], F32, tag="red")
            bounds = [0, QW, 2 * QW, 3 * QW, GTW]
            for h in range(2):
                hs = slice(h * HW, (h + 1) * HW)
                nc.vector.tensor_scalar(rx[:, hs], zx[:, hs], 0.0, None, Alu.max)
                nc.vector.tensor_scalar(ry[:, hs], zy[:, hs], 0.0, None, Alu.max)
                nc.vector.tensor_mul(inter[:, hs], rx[:, hs], ry[:, hs])
                for qq in range(2):
                    hq = 2 * h + qq
                    sl = slice(bounds[hq], bounds[hq + 1])
                    nc.scalar.activation(lnI[:, sl], inter[:, sl], Act.Ln)
                    nc.vector.tensor_sub(lnd[:, sl], lnI[:, sl], lnS[:, sl])
                    nc.vector.tensor_reduce(
                        red[:, hq : hq + 1], lnd[:, sl],
                        mybir.AxisListType.X, Alu.max,
                    )

            # ------- union max in the gtb DMA-wait bubble on DVE ------------
            msk = small.tile([128, 32], F16, tag="msk")
            nc.vector.tensor_scalar(msk[:], ucls[:], 0.0, None, Alu.is_equal)
            sm = small.tile([128, 32], F16, tag="sm")
            nc.vector.tensor_mul(sm[:], msk[:], uscore[:])
            nc.vector.tensor_reduce(
                red[:, 4:5], sm[:], mybir.AxisListType.X, Alu.max
            )

            # ---- out: [128, 5] = per-partition quarter maxima + umax -------
            nc.sync.dma_start(out=out.ap(), in_=red[:])

    nc.compile()
    return nc


_KERNEL_CACHE = {}

# test/dev hooks
TRACE = False
LAST_RESULTS = None


def _get_kernel(K: int):
    if K not in _KERNEL_CACHE:
        _KERNEL_CACHE[K] = build_kernel(K)
    return _KERNEL_CACHE[K]


def make_in_maps(pred_boxes, pred_classes, gt_boxes, union_scores, union_classes, K):
    nhalf = 128 // K
    GTW = M // nhalf
    f16 = np.float16
    uscore16 = union_scores.astype(f16).reshape(128, 32)
    ucls16 = union_classes.astype(f16).reshape(128, 32)
    # iota: [128, 128], values (j % K) + 1 on every row
    iota = np.broadcast_to(
        np.tile(np.arange(1, K + 1, dtype=f16), 128 // K), (128, 128)
    )
    tri = (np.arange(128)[:, None] < np.arange(128)[None, :]).astype(f16)

    def pack_u32(a16):
        a16 = np.ascontiguousarray(a16)
        u16 = a16.view(np.uint16).astype(np.uint32)
        return u16[:, 0::2] | (u16[:, 1::2] << 16)

    iota_u32 = pack_u32(iota)  # [128, iw//2]
    tri_u32 = pack_u32(tri)  # [128, 64]
    us_u32 = pack_u32(uscore16)
    uc_u32 = pack_u32(ucls16)

    in_maps = []
    for b in range(B):
        pbc = (pred_boxes[b] - CEN).astype(f16).reshape(128, 128)  # 32 chunks x 4
        cls_u32 = pred_classes[b].reshape(128, 32).view(np.uint32)
        cols = [cls_u32, pack_u32(pbc), us_u32, uc_u32, tri_u32, iota_u32]
        bigarr = np.concatenate(cols, axis=1)
        assert bigarr.shape[1] == 256, bigarr.shape
        gtc = (gt_boxes[b] - CEN).astype(f16)  # [M, 4]
        ag = (
            (gt_boxes[b][:, 2] - gt_boxes[b][:, 0])
            * (gt_boxes[b][:, 3] - gt_boxes[b][:, 1])
        ).astype(f16)
        # column planes per half: [nhalf, 5*GTW] = x1|x2|y1|y2|area
        gtc = gtc.reshape(nhalf, GTW, 4)
        gt_planes = np.concatenate(
            [gtc[:, :, 0], gtc[:, :, 2], gtc[:, :, 1], gtc[:, :, 3],
             ag.reshape(nhalf, GTW)],
            axis=1,
        )
        in_maps.append(
            {
                "big": np.ascontiguousarray(bigarr.astype(np.uint32)),
                "gt_cols": np.ascontiguousarray(gt_planes),
            }
        )
    return in_maps


def kernel(pred_boxes, pred_scores, pred_classes, gt_boxes, union_scores, union_classes):
    from concourse.bass_utils import run_bass_kernel_spmd

    pred_boxes = np.ascontiguousarray(np.asarray(pred_boxes, dtype=np.float32))
    pred_classes = np.ascontiguousarray(np.asarray(pred_classes, dtype=np.int32))
    gt_boxes = np.ascontiguousarray(np.asarray(gt_boxes, dtype=np.float32))
    union_scores = np.ascontiguousarray(np.asarray(union_scores, dtype=np.float32))
    union_classes = np.ascontiguousarray(np.asarray(union_classes, dtype=np.int32))

    max_persons = int((pred_classes == 0).sum(axis=1).max())
    K = 64 if max_persons <= 64 else 128
    nc = _get_kernel(K)

    in_maps = make_in_maps(
        pred_boxes, pred_classes, gt_boxes, union_scores, union_classes, K
    )
    res = run_bass_kernel_spmd(nc, in_maps, list(range(B)), trace=TRACE)
    global LAST_RESULTS
    LAST_RESULTS = res
    outs = np.stack([res.results[b]["out"] for b in range(B)])  # [B, 128, 5]
    max_prob = outs[0, :, 4].max()
    lmax = outs[:, :, 0:4].max(axis=(1, 2))  # [B] per-image max ln(inter/S)
    r = np.exp(lmax.astype(np.float64))
    iou = r / (1.0 - r)  # ln-rank back to iou = r/(1-r)
    max_iou = np.float32(iou.mean())
    return np.array([max_prob, max_iou], dtype=np.float32)


# revision 29
# speedup vs baseline: 1.0992x; 1.0992x over previous
"""Trainium2 Bass kernel for nn_MaxExtractor (masked pairwise-IoU max + union max).

Contract: kernel(**inputs) takes FULL unsharded inputs, returns the FULL [2]
output. Internally shards the batch dim (8 images) across 8 NeuronCores, one
image per core; each core computes [max_prob_t, iou_max_of_its_image]; the
host gathers and averages the per-image iou scalars.

v2 design (per core, N=4096 preds, M=2048 gts, K=64 person slots):
  - All coordinates are shipped as fp16, centered by -IMG/2 so the fp16
    quantization step is <=0.25 over the whole canvas.
  - gt coord column-planes live in DRAM as [nhalf, 4*GTW]; ONE broadcast DMA
    (stride-0 partition dim) replicates each half across 64 partitions
    directly into SBUF [128, 4*GTW] fp16 - no PE broadcast, no PSUM.
  - Person compaction: class mask -> free-dim cumsum -> cross-partition
    prefix (triangular fp16 matmul) -> ranks -> 32x one-hot (fp16, 4x DVE
    mode) -> accumulating PE matmuls gather person boxes into [128, 4].
  - Pairwise runs all-SBUF fp16 (tensor_scalar at 4x, tensor_tensor at 2x):
    t1x = min(gx2,px2), m2x = max(gx1,px1), zx = t1x-m2x, same for y,
    inter = relu(zx)*relu(zy).
  - Ranking in log domain on the otherwise-idle Act engine:
    lnd = Ln(inter) - Ln(area_p + area_g), max-reduced; the single winner is
    mapped back via r = exp(lnd), iou = r/(1-r).  Areas for gt come from the
    broadcast planes (3 fp16 tensor_tensor ops).
  - Union max: fp16 mask-multiply + reduce.
  - Final: gpsimd partition all-reduce(max) over [umax | iou], 1 output DMA.
"""

import sys

sys.path.insert(0, "/opt/trn_rl_repo")

import contextlib

import numpy as np

import concourse.bacc as bacc
import concourse.mybir as mybir
from concourse import bass_isa
from concourse.tile import TileContext

F32 = mybir.dt.float32
F16 = mybir.dt.float16
U32 = mybir.dt.uint32
I32 = mybir.dt.int32
Alu = mybir.AluOpType
Act = mybir.ActivationFunctionType

N = 4096  # preds per image
M = 2048  # gts per image
B = 8  # images == cores
U = 4096  # union entries
CEN = 320.0  # coordinate centering offset (IMG/2)
EPS = 1.0e-9
NCH = 32  # pred chunks of 128 (compaction contract dim)
NPOOL_OH = 8  # one-hot chunks built on Pool (rest on DVE)


def build_kernel(K: int):
    """Build the per-core Bass module. K = person-slot count (64 or 128)."""
    assert K in (64, 128)
    nhalf = 128 // K  # gt halves packed along partitions
    GTW = M // nhalf  # gt columns per partition (1024 for K=64)
    HW = GTW // 2  # half-width for the pipelined ln/reduce stage

    nc = bacc.Bacc("TRN2", target_bir_lowering=False, debug=False)

    # host-packed inputs
    #   big u32 [128, 256]: cls(32) | pb_f16(64) | uscore_f16(16) | ucls_f16(16)
    #                      | tri_f16(64) | iota_f16(64)
    #   gt_cols f16 [nhalf, 5*GTW]: x1 | x2 | y1 | y2 | area planes
    big = nc.dram_tensor("big", [128, 256], U32, kind="ExternalInput")
    gt_cols = nc.dram_tensor("gt_cols", [nhalf, 5 * GTW], F16, kind="ExternalInput")
    out = nc.dram_tensor("out", [128, 5], F32, kind="ExternalOutput")

    with TileContext(nc) as tc:
        ctx = contextlib.ExitStack()
        with ctx:
            sb = ctx.enter_context(tc.tile_pool(name="sbuf", bufs=1))
            wrk = ctx.enter_context(tc.tile_pool(name="wrk", bufs=2))
            ohp = ctx.enter_context(tc.tile_pool(name="ohp", bufs=32))
            small = ctx.enter_context(tc.tile_pool(name="small", bufs=1))
            ps_s = ctx.enter_context(tc.tile_pool(name="ps_s", bufs=2, space="PSUM"))

            # ---------------- loads ----------------
            bigt = sb.tile([128, 256], U32, tag="bigt")
            nc.sync.dma_start(out=bigt[:], in_=big.ap())
            cls_sb = bigt[:, 0:32].bitcast(I32)
            pb = bigt[:, 32:96].bitcast(F16)  # [128, 128] = 32 chunks x 4
            uscore = bigt[:, 96:112].bitcast(F16)  # [128, 32]
            ucls = bigt[:, 112:128].bitcast(F16)  # [128, 32]
            tri_sb = bigt[:, 128:192].bitcast(F16)  # [128, 128]
            iota_sb = bigt[:, 192:256].bitcast(F16)  # [128, 128], (j % K) + 1

            # gt broadcast: partition p reads half p // K of gt_cols.
            # Coord planes and the area plane ship as separate DMAs so the
            # pairwise chain can start before the area transfer completes.
            gtb = sb.tile([128, 5 * GTW], F16, tag="gtb")
            src_xy = gt_cols.ap()[:, 0 : 4 * GTW].unsqueeze(1)
            nc.sync.dma_start(
                out=gtb[:, 0 : 4 * GTW],
                in_=src_xy.broadcast_to([nhalf, K, 4 * GTW]),
            )
            src_ag = gt_cols.ap()[:, 4 * GTW : 5 * GTW].unsqueeze(1)
            nc.sync.dma_start(
                out=gtb[:, 4 * GTW : 5 * GTW],
                in_=src_ag.broadcast_to([nhalf, K, GTW]),
            )

            # preload the Ln activation table while DMAs are in flight: a
            # dummy Ln on a memset tile adopts the table-load so the real
            # lnS/lnI calls don't pay the 1283ns load on the critical path
            dmy = small.tile([128, 1], F32, tag="dmy")
            nc.vector.memset(dmy[:], 1.0)
            dmy2 = small.tile([128, 1], F32, tag="dmy2")
            nc.scalar.activation(dmy2[:], dmy[:], Act.Ln)
            gx1 = gtb[:, 0 * GTW : 1 * GTW]
            gx2 = gtb[:, 1 * GTW : 2 * GTW]
            gy1 = gtb[:, 2 * GTW : 3 * GTW]
            gy2 = gtb[:, 3 * GTW : 4 * GTW]
            agb = gtb[:, 4 * GTW : 5 * GTW]

            # ---------------- person mask + ranks (DVE) ----------------
            m = small.tile([128, 32], F16, tag="m")
            nc.vector.tensor_scalar(m[:], cls_sb[:], 0, None, Alu.is_equal)
            s = small.tile([128, 32], F16, tag="s")
            nc.vector.tensor_tensor_scan(s[:], m[:], m[:], 0.0, Alu.add, Alu.max)
            pref_ps = ps_s.tile([128, 4], F32, tag="pss")
            nc.tensor.matmul(
                pref_ps[:, 0:1], tri_sb, s[:, 31:32], start=True, stop=True
            )
            q = small.tile([128, 32], F32, tag="q")
            nc.vector.scalar_tensor_tensor(
                q[:], s[:], pref_ps[:, 0:1], m[:], Alu.add, Alu.mult
            )

            # ---------------- compaction: one-hot + matmul gather -----------
            # oh[p, j] = (q[p, f] == iota[j]), iota[j] = (j % K) + 1
            # Last NPOOL_OH chunks build on Pool so PE's in-order accumulation
            # is never stalled by the slower Pool ops.
            pc_ps = ps_s.tile([128, 4], F32, tag="pss")
            for f in range(NCH):
                oh = ohp.tile([128, 128], F16, tag="oh")
                eng = nc.gpsimd if f < NPOOL_OH else nc.vector
                eng.tensor_scalar(
                    oh[:], iota_sb, q[:, f : f + 1], None, Alu.is_equal
                )
                nc.tensor.matmul(
                    pc_ps[:], oh[:], pb[:, 4 * f : 4 * f + 4],
                    start=(f == 0), stop=(f == NCH - 1),
                )
            pc = small.tile([128, 4], F32, tag="pcs")
            nc.vector.tensor_copy(pc[:], pc_ps[:])
            px1, py1, px2, py2 = (pc[:, i : i + 1] for i in range(4))
            wp = small.tile([128, 1], F32, tag="wp")
            nc.vector.tensor_sub(wp[:], px2, px1)
            hp = small.tile([128, 1], F32, tag="hp")
            nc.vector.tensor_sub(hp[:], py2, py1)
            ap_eps = small.tile([128, 1], F32, tag="ap_eps")
            nc.vector.scalar_tensor_tensor(
                ap_eps[:], wp[:], EPS, hp[:], Alu.bypass, Alu.mult
            )
            nc.vector.tensor_scalar(ap_eps[:], ap_eps[:], EPS, None, Alu.add)

            # lnS = Ln(agb + area_p) on Act, in quarters for pipelining
            QW = GTW // 4
            lnS = wrk.tile([128, GTW], F16, tag="lnS")
            for hq in range(4):
                sl = slice(hq * QW, (hq + 1) * QW)
                nc.scalar.activation(
                    lnS[:, sl], agb[:, sl], Act.Ln, bias=ap_eps[:], scale=1.0
                )

            # ---------------- pairwise intersection (fp16, all SBUF) --------
            t1x = wrk.tile([128, GTW], F16, tag="t1x")
            nc.vector.tensor_scalar(t1x[:], gx2, px2, None, Alu.min)
            m2x = wrk.tile([128, GTW], F16, tag="m2x")
            nc.vector.tensor_scalar(m2x[:], gx1, px1, None, Alu.max)
            zx = wrk.tile([128, GTW], F16, tag="zx")
            nc.vector.tensor_sub(zx[:], t1x[:], m2x[:])
            t1y = wrk.tile([128, GTW], F16, tag="t1y")
            nc.vector.tensor_scalar(t1y[:], gy2, py2, None, Alu.min)
            m2y = wrk.tile([128, GTW], F16, tag="m2y")
            nc.vector.tensor_scalar(m2y[:], gy1, py1, None, Alu.max)
            zy = wrk.tile([128, GTW], F16, tag="zy")
            nc.vector.tensor_sub(zy[:], t1y[:], m2y[:])
            # rx/ry/inter per half so the Act ln chain starts earlier;
            # lnI/lnd/reduce per quarter to pipeline Act against DVE
            rx = wrk.tile([128, GTW], F16, tag="rx")
            ry = wrk.tile([128, GTW], F16, tag="ry")
            inter = wrk.tile([128, GTW], F16, tag="inter")
            lnI = wrk.tile([128, GTW], F16, tag="lnI")
            lnd = wrk.tile([128, GTW], F16, tag="lnd")
            # red cols 0-3: lnd quarter maxima; col 4: union max
            red = small.tile([128, 5], F32, tag="red")
            bounds = [0, QW, 2 * QW, 3 * QW, GTW]
            for h in range(2):
                hs = slice(h * HW, (h + 1) * HW)
                nc.vector.tensor_scalar(rx[:, hs], zx[:, hs], 0.0, None, Alu.max)
                nc.vector.tensor_scalar(ry[:, hs], zy[:, hs], 0.0, None, Alu.max)
                nc.vector.tensor_mul(inter[:, hs], rx[:, hs], ry[:, hs])
                for qq in range(2):
                    hq = 2 * h + qq
                    sl = slice(bounds[hq], bounds[hq + 1])
                    nc.scalar.activation(lnI[:, sl], inter[:, sl], Act.Ln)
                    nc.vector.tensor_sub(lnd[:, sl], lnI[:, sl], lnS[:, sl])
                    nc.vector.tensor_reduce(
                        red[:, hq : hq + 1], lnd[:, sl],
                        mybir.AxisListType.X, Alu.max,
                    )

            # ------- union max in the gtb DMA-wait bubble on DVE ------------
            msk = small.tile([128, 32], F16, tag="msk")
            nc.vector.tensor_scalar(msk[:], ucls[:], 0.0, None, Alu.is_equal)
            sm = small.tile([128, 32], F16, tag="sm")
            nc.vector.tensor_mul(sm[:], msk[:], uscore[:])
            nc.vector.tensor_reduce(
                red[:, 4:5], sm[:], mybir.AxisListType.X, Alu.max
            )

            # ---- out: [128, 5] = per-partition quarter maxima + umax -------
            nc.sync.dma_start(out=out.ap(), in_=red[:])

    nc.compile()
    return nc


_KERNEL_CACHE = {}

# test/dev hooks
TRACE = False
LAST_RESULTS = None


def _get_kernel(K: int):
    if K not in _KERNEL_CACHE:
        _KERNEL_CACHE[K] = build_kernel(K)
    return _KERNEL_CACHE[K]


def make_in_maps(pred_boxes, pred_classes, gt_boxes, union_scores, union_classes, K):
    nhalf = 128 // K
    GTW = M // nhalf
    f16 = np.float16
    uscore16 = union_scores.astype(f16).reshape(128, 32)
    ucls16 = union_classes.astype(f16).reshape(128, 32)
    # iota: [128, 128], values (j % K) + 1 on every row
    iota = np.broadcast_to(
        np.tile(np.arange(1, K + 1, dtype=f16), 128 // K), (128, 128)
    )
    tri = (np.arange(128)[:, None] < np.arange(128)[None, :]).astype(f16)

    def pack_u32(a16):
        a16 = np.ascontiguousarray(a16)
        u16 = a16.view(np.uint16).astype(np.uint32)
        return u16[:, 0::2] | (u16[:, 1::2] << 16)

    iota_u32 = pack_u32(iota)  # [128, iw//2]
    tri_u32 = pack_u32(tri)  # [128, 64]
    us_u32 = pack_u32(uscore16)
    uc_u32 = pack_u32(ucls16)

    in_maps = []
    for b in range(B):
        pbc = (pred_boxes[b] - CEN).astype(f16).reshape(128, 128)  # 32 chunks x 4
        cls_u32 = pred_classes[b].reshape(128, 32).view(np.uint32)
        cols = [cls_u32, pack_u32(pbc), us_u32, uc_u32, tri_u32, iota_u32]
        bigarr = np.concatenate(cols, axis=1)
        assert bigarr.shape[1] == 256, bigarr.shape
        gtc = (gt_boxes[b] - CEN).astype(f16)  # [M, 4]
        ag = (
            (gt_boxes[b][:, 2] - gt_boxes[b][:, 0])
            * (gt_boxes[b][:, 3] - gt_boxes[b][:, 1])
        ).astype(f16)
        # column planes per half: [nhalf, 5*GTW] = x1|x2|y1|y2|area
        gtc = gtc.reshape(nhalf, GTW, 4)
        gt_planes = np.concatenate(
            [gtc[:, :, 0], gtc[:, :, 2], gtc[:, :, 1], gtc[:, :, 3],
             ag.reshape(nhalf, GTW)],
            axis=1,
        )
        in_maps.append(
            {
                "big": np.ascontiguousarray(bigarr.astype(np.uint32)),
                "gt_cols": np.ascontiguousarray(gt_planes),
            }
        )
    return in_maps


def kernel(pred_boxes, pred_scores, pred_classes, gt_boxes, union_scores, union_classes):
    from concourse.bass_utils import run_bass_kernel_spmd

    pred_boxes = np.ascontiguousarray(np.asarray(pred_boxes, dtype=np.float32))
    pred_classes = np.ascontiguousarray(np.asarray(pred_classes, dtype=np.int32))
    gt_boxes = np.ascontiguousarray(np.asarray(gt_boxes, dtype=np.float32))
    union_scores = np.ascontiguousarray(np.asarray(union_scores, dtype=np.float32))
    union_classes = np.ascontiguousarray(np.asarray(union_classes, dtype=np.int32))

    max_persons = int((pred_classes == 0).sum(axis=1).max())
    K = 64 if max_persons <= 64 else 128
    nc = _get_kernel(K)

    in_maps = make_in_maps(
        pred_boxes, pred_classes, gt_boxes, union_scores, union_classes, K
    )
    res = run_bass_kernel_spmd(nc, in_maps, list(range(B)), trace=TRACE)
    global LAST_RESULTS
    LAST_RESULTS = res
    outs = np.stack([res.results[b]["out"] for b in range(B)])  # [B, 128, 5]
    max_prob = outs[0, :, 4].max()
    lmax = outs[:, :, 0:4].max(axis=(1, 2))  # [B] per-image max ln(inter/S)
    r = np.exp(lmax.astype(np.float64))
    iou = r / (1.0 - r)  # ln-rank back to iou = r/(1-r)
    max_iou = np.float32(iou.mean())
    return np.array([max_prob, max_iou], dtype=np.float32)
